# revision 1
# baseline (speedup 1.0000x reference)
"""Trainium2 Bass kernel for nn_MultiHeadAttention (B=4, S=2048, E=1024, H=16, D=64).

Sharding: 8 cores = 4 batches x 2 head-halves. Core c handles batch c//2 and
heads [ (c%2)*8, (c%2)*8+8 ). Each core computes its heads' attention and a
partial output projection; the host sums the two per-batch partials and adds bo.

ScalarE exp over the 33.5M score elements per core is the critical resource;
the whole program is ordered so it starts within ~15us and never stalls:
  - All projections are software-pipelined INTO the first q-chunk's attention
    loop: scores(t, chunk) only needs krot[t]'s chunk, attnv(t, chunk) only
    needs vaug's chunk, so K/V projection pieces are emitted chunk-by-chunk
    just ahead of their consumers; Q projection for chunk qc+1 and the
    output projection for qc-1 fill the PE between d-tiles afterwards.
  - Scores contract K=64 per head; the two heads of a d-tile run as row-tiled
    pairs (tile_position (0,0)/(64,0)) into the two banks of one [P,2,512]
    PSUM tile. One ScalarE activation evicts both banks: exp(0.125*scores)
    fp32-PSUM -> bf16 SBUF with the scale fused (no staging op).
  - attn_out^T[d,q] = V_aug.T @ P^T with V_aug = [V | ones] (col 64 = softmax
    denominator). The attnv accumulation trails the scores stream by one
    4-kt chunk so the PE alternation (and its 64/128 row-mode switches) stays
    coarse while ScalarE keeps draining.
  - Normalization: one batched DVE reciprocal of both heads' denominator
    rows + GpSimd partition broadcast + two muls into the bf16 aall tile.
  - RoPE is fused into every projection eviction via a bf16 staging copy
    (pairs at partition distance 32; sign baked into the sin table).
  - Inputs arrive via combined strided DMAs ordered by first consumption.
"""

import os
import sys
import numpy as np

sys.path.insert(0, "/opt/trn_rl_repo")

from contextlib import ExitStack

import concourse.bacc as bacc
import concourse.tile as tile
from concourse import mybir
from concourse.bass_utils import run_bass_kernel_spmd

B, S, E = 4, 2048, 1024
H, D = 16, 64
HPC = 8          # heads per core
DPC = HPC * D    # 512 d-dims per core
P = 128
NSC = S // 512   # 4 s-chunks of 512
NST = S // 128   # 16 s-tiles of 128
NET = E // 128   # 8 e-tiles of 128
NDT = DPC // 128  # 4 d-tiles of 128 (= head pairs)
NKQ = 4          # kt tiles per pt chunk

F32 = mybir.dt.float32
BF16 = mybir.dt.bfloat16

REPEAT = int(os.environ.get("KERNEL_REPEAT", "1"))
SCORES_PACK = os.environ.get("SCORES_PACK", "1") == "1"
EXP_PATH = os.environ.get("EXP_PATH", "direct")  # direct | staged
PT_BUFS = int(os.environ.get("PT_BUFS", "4"))


def build_program(repeat=None):
    global REPEAT
    if repeat is not None:
        REPEAT = repeat
    nc = bacc.Bacc("TRN2", target_bir_lowering=False, debug=False, num_devices=8)

    qT = nc.dram_tensor("qT", [E, S], BF16, kind="ExternalInput").ap()
    kT = nc.dram_tensor("kT", [E, S], BF16, kind="ExternalInput").ap()
    vT = nc.dram_tensor("vT", [E, S], BF16, kind="ExternalInput").ap()
    wqT = nc.dram_tensor("wqT", [E, DPC], BF16, kind="ExternalInput").ap()
    wkT = nc.dram_tensor("wkT", [E, DPC], BF16, kind="ExternalInput").ap()
    wvT = nc.dram_tensor("wvT", [E, DPC], BF16, kind="ExternalInput").ap()
    woT = nc.dram_tensor("woT", [DPC, E], BF16, kind="ExternalInput").ap()
    ctab = nc.dram_tensor("ctab", [P, S], BF16, kind="ExternalInput").ap()
    stab = nc.dram_tensor("stab", [P, S], BF16, kind="ExternalInput").ap()
    out = nc.dram_tensor("out", [S, E], F32, kind="ExternalOutput").ap()

    with tile.TileContext(nc) as tc:
        with ExitStack() as ctx:
            body(ctx, tc, nc, qT, kT, vT, wqT, wkT, wvT, woT, ctab, stab, out)
    nc.compile()
    return nc


def body(ctx, tc, nc, qT, kT, vT, wqT, wkT, wvT, woT, ctab, stab, out):
    consts = ctx.enter_context(tc.tile_pool(name="consts", bufs=1))
    c_sb = consts.tile([P, S], BF16, tag="ctab")
    s_sb = consts.tile([P, S], BF16, tag="stab")

    wpool = ctx.enter_context(tc.tile_pool(name="wpool", bufs=1))
    wopool = ctx.enter_context(tc.tile_pool(name="wopool", bufs=1))
    ktpool = ctx.enter_context(tc.tile_pool(name="ktpool", bufs=1))
    xq = ctx.enter_context(tc.tile_pool(name="xq", bufs=1))
    xv = ctx.enter_context(tc.tile_pool(name="xv", bufs=1))
    vt0_pool = ctx.enter_context(tc.tile_pool(name="vt0", bufs=1))
    xq0_pool = ctx.enter_context(tc.tile_pool(name="xq0", bufs=1))

    krot_pool = ctx.enter_context(tc.tile_pool(name="krot", bufs=4))
    qrot_pool = ctx.enter_context(tc.tile_pool(name="qrot", bufs=2))
    vaug_pool = ctx.enter_context(tc.tile_pool(name="vaug", bufs=1))
    aall_pool = ctx.enter_context(tc.tile_pool(name="aall", bufs=2))
    pt_pool = ctx.enter_context(tc.tile_pool(name="pt", bufs=PT_BUFS))

    rtmp = ctx.enter_context(tc.tile_pool(name="rtmp", bufs=2))
    stg = (ctx.enter_context(tc.tile_pool(name="stg", bufs=2))
           if EXP_PATH == "staged" else None)
    ntmp = ctx.enter_context(tc.tile_pool(name="ntmp", bufs=1))
    opool = ctx.enter_context(tc.tile_pool(name="opool", bufs=2))

    psum_s = ctx.enter_context(tc.tile_pool(name="psum_s", bufs=2, space="PSUM"))
    psum_av = ctx.enter_context(tc.tile_pool(name="psum_av", bufs=1, space="PSUM"))
    psum_p = ctx.enter_context(tc.tile_pool(name="psum_p", bufs=2, space="PSUM"))

    # weights + kT resident for the whole kernel, loaded with one combined
    # strided DMA per tensor (SP dispatch of many small dma_starts would
    # otherwise gate the pipeline head). Issue order tracks consumption:
    # the first exp only needs wk + kT(sc0) + wq + qT(qc0) + tables (~5MB).
    # DMA transfer time is charged serially on the ISSUING engine, so the
    # input loads are split between SP (sync) and the nearly-idle GpSimd
    # queue: two parallel chains instead of one serial one.
    def load_etiles(pool, tag, dram, cols, n=NET, eng=None):
        t_ = pool.tile([P, n, cols], BF16, tag=tag, name=tag)
        (eng or nc.sync).dma_start(out=t_[:],
                                   in_=dram.rearrange("(a p) c -> p a c", p=P))
        return t_

    wk_a = load_etiles(wpool, "wk", wkT, DPC)
    ktc_a = ktpool.tile([P, NSC, NET, 512], BF16, tag="kt", name="ktc")
    nc.gpsimd.dma_start(out=ktc_a[:, 0],
                        in_=kT[:, 0:512].rearrange("(a p) c -> p a c", p=P))
    wq_a = load_etiles(wpool, "wq", wqT, DPC, eng=nc.gpsimd)
    xq0 = load_etiles(xq0_pool, "xq0", qT[:, 0:512], 512)
    # only the first 512 table columns gate the first RoPE eviction
    nc.sync.dma_start(out=c_sb[:, 0:512], in_=ctab[:, 0:512])
    nc.gpsimd.dma_start(out=s_sb[:, 0:512], in_=stab[:, 0:512])
    nc.sync.dma_start(out=c_sb[:, 512:], in_=ctab[:, 512:])
    nc.gpsimd.dma_start(out=s_sb[:, 512:], in_=stab[:, 512:])
    nc.sync.dma_start(out=ktc_a[:, 1],
                      in_=kT[:, 512:1024].rearrange("(a p) c -> p a c", p=P))
    wv_a = load_etiles(wpool, "wv", wvT, DPC, eng=nc.gpsimd)
    vt0 = load_etiles(vt0_pool, "vt0", vT[:, 0:512], 512)
    wo_a = load_etiles(wopool, "wo", woT, E, n=NDT, eng=nc.gpsimd)
    wk_sb = [wk_a[:, et, :] for et in range(NET)]
    wq_sb = [wq_a[:, et, :] for et in range(NET)]
    wv_sb = [wv_a[:, et, :] for et in range(NET)]
    wo_sb = [wo_a[:, t, :] for t in range(NDT)]
    ktc = [[ktc_a[:, sc, et, :] for sc in range(NSC)] for et in range(NET)]
    xq0 = [xq0[:, et, :] for et in range(NET)]
    vt0 = [vt0[:, et, :] for et in range(NET)]

    pools = (c_sb, s_sb, wk_sb, wq_sb, wv_sb, wo_sb, ktc, ktc_a, xq, xv, xq0, vt0,
             krot_pool, qrot_pool, vaug_pool, aall_pool, pt_pool,
             rtmp, stg, ntmp, opool, psum_s, psum_av, psum_p)
    for rep in range(REPEAT):
        one_pass(tc, nc, qT, kT, vT, out, *pools)


def one_pass(tc, nc, qT, kT, vT, out,
             c_sb, s_sb, wk_sb, wq_sb, wv_sb, wo_sb, ktc, ktc_a, xq, xv, xq0, vt0,
             krot_pool, qrot_pool, vaug_pool, aall_pool, pt_pool,
             rtmp, stg, ntmp, opool, psum_s, psum_av, psum_p):

    def rope_evict(ps, dst, ssl):
        """ps [P,512] f32 PSUM -> dst [P,512] bf16 rotated, via bf16 staging.

        Row layout per 64 rows (one head): [32 evens | 32 odds]; RoPE pairs
        sit at partition distance 32, so the swap is between 32-blocks
        (0<->1, 2<->3). s_sb rows carry the sign: [-sin, sin, -sin, sin].
        """
        st_bf = rtmp.tile([P, 512], BF16, tag="st", name="st_bf")
        nc.vector.tensor_copy(st_bf[:], ps[:])
        xsw = rtmp.tile([P, 512], BF16, tag="xsw", name="xsw")
        for blk in range(4):
            sb = blk ^ 1
            nc.vector.tensor_copy(xsw[blk * 32:(blk + 1) * 32, :],
                                  st_bf[sb * 32:(sb + 1) * 32, :])
        nc.vector.tensor_mul(xsw[:], xsw[:], s_sb[:, ssl])
        t2 = rtmp.tile([P, 512], BF16, tag="t2", name="t2")
        nc.vector.tensor_mul(t2[:], st_bf[:], c_sb[:, ssl])
        nc.vector.tensor_add(dst, t2[:], xsw[:])

    def proj_chunk(w_sb, x_tiles, t, nm):
        ps = psum_p.tile([P, 512], F32, tag="pp", name=f"pp_{nm}")
        for et in range(NET):
            nc.tensor.matmul(
                ps[:], w_sb[et][:, t * P:(t + 1) * P], x_tiles[et],
                start=(et == 0), stop=(et == NET - 1),
            )
        return ps

    krot = [krot_pool.tile([P, S], BF16, tag="krot", name=f"krot{i}")
            for i in range(NDT)]
    vaug = vaug_pool.tile([P, NST, HPC * 65], BF16, tag="vaug")
    vaug_v = vaug.rearrange("p st (h dd) -> p st h dd", h=HPC)
    nc.vector.memset(vaug_v[:, :, :, 64:65], 1.0)

    def kproj_piece(t, sc):
        ssl = slice(sc * 512, (sc + 1) * 512)
        ps = proj_chunk(wk_sb, [ktc[et][sc] for et in range(NET)],
                        t, f"k{t}{sc}")
        rope_evict(ps, krot[t][:, ssl], ssl)

    qp_state = {}

    def qproj_load(qc):
        qsl = slice(qc * 512, (qc + 1) * 512)
        xq_a = xq.tile([P, NET, 512], BF16, tag="xqp", name=f"x_q{qc}")
        nc.gpsimd.dma_start(out=xq_a[:],
                          in_=qT[:, qsl].rearrange("(a p) c -> p a c", p=P))
        qr = qrot_pool.tile([P, NDT, 512], BF16, tag="qrot", name=f"qr{qc}")
        qp_state[qc] = (xq_a, qr)
        return qr

    def qproj_piece(qc, t):
        xq_a, qr = qp_state[qc]
        qsl = slice(qc * 512, (qc + 1) * 512)
        ps = proj_chunk(wq_sb, [xq_a[:, et, :] for et in range(NET)],
                        t, f"q{qc}{t}")
        rope_evict(ps, qr[:, t, :], qsl)

    vload_pending = {}

    def vload(sc):
        # prefetch the vT chunk one scores-chunk ahead of its vproj
        xv_a = xv.tile([P, NET, 512], BF16, tag="xv", name=f"x_v{sc}")
        nc.gpsimd.dma_start(
            out=xv_a[:],
            in_=vT[:, sc * 512:(sc + 1) * 512].rearrange(
                "(a p) c -> p a c", p=P))
        vload_pending[sc] = xv_a

    def vproj_full(sc):
        # V for all 8 heads over the 4 s-tiles of chunk sc (N=512 matmuls)
        if sc == 0:
            xv_sb = list(vt0)
        else:
            xv_a = vload_pending.pop(sc)
            xv_sb = [xv_a[:, et, :] for et in range(NET)]
        for sti in range(4):
            st = sc * 4 + sti
            ps = psum_p.tile([P, 512], F32, tag="pp", name=f"pp_v{sc}{st}")
            for et in range(NET):
                nc.tensor.matmul(ps[:],
                                 xv_sb[et][:, sti * P:(sti + 1) * P],
                                 wv_sb[et][:],
                                 start=(et == 0), stop=(et == NET - 1))
            nc.vector.tensor_copy(
                vaug_v[:, st, :, 0:64],
                ps[:].rearrange("p (h d) -> p h d", h=HPC),
            )

    def outproj_st(qc, aall, sti):
        # output projection for one s-tile of chunk qc
        if True:
            st = qc * 4 + sti
            osb = opool.tile([P, E], F32, tag="osb", name=f"osb{st}")
            for ec in range(2):
                esl = slice(ec * 512, (ec + 1) * 512)
                ps_f = psum_p.tile([P, 512], F32, tag="pp", name=f"pp_f{st}{ec}")
                for t in range(NDT):
                    nc.tensor.matmul(ps_f[:],
                                     aall[:, t, sti * P:(sti + 1) * P],
                                     wo_sb[t][:, esl],
                                     start=(t == 0), stop=(t == NDT - 1))
                nc.vector.tensor_copy(osb[:, esl], ps_f[:])
            nc.sync.dma_start(out=out[st * P:(st + 1) * P, :], in_=osb[:])

    # ---------------- attention, software-pipelined per q-chunk ----------------
    # qc0 prologue: qT chunk-0 tiles + the first Q/K projection pieces.
    # K and V projections for tile t are pipelined chunk-by-chunk into qc0's
    # scores stream (scores(t, ktq) only needs krot[t] chunk ktq).
    qr0 = qrot_pool.tile([P, NDT, 512], BF16, tag="qrot", name="qr0")

    def qproj0_piece(t):
        ps = proj_chunk(wq_sb, list(xq0), t, f"q0{t}")
        rope_evict(ps, qr0[:, t, :], slice(0, 512))

    kproj_piece(0, 0)
    qproj0_piece(0)

    qr_cur = qr0
    aall_prev = None
    aall = None
    pending = None  # (t, ktq, ptt, aall) attnv chunk awaiting emission
    av_state = {"ps_o": None}

    def emit_attnv_part(t, ktq, ptt, j):
        # two matmuls (both heads) for one kt of a pending chunk
        if ktq == 0 and j == 0:
            av_state["ps_o"] = psum_av.tile([P, 2, 512], F32, tag="po",
                                            name=f"po{t}")
        ps_o = av_state["ps_o"]
        kt = ktq * NKQ + j
        for gl in range(2):
            g = 2 * t + gl
            nc.tensor.matmul(ps_o[0:65, gl, :],
                             vaug[:, kt, g * 65:(g + 1) * 65],
                             ptt[:, j, gl, :],
                             start=(kt == 0), stop=(kt == NST - 1))

    def emit_attnv(t, ktq, ptt, aall, skip_parts=0):
        ps_o = av_state["ps_o"]
        for j in range(skip_parts, NKQ):
            emit_attnv_part(t, ktq, ptt, j)
        if ktq == NST // NKQ - 1:
            # normalize: batched reciprocal of both denominator rows,
            # broadcast across the 64 d partitions, scale both heads
            rec = ntmp.tile([1, 2, 512], F32, tag="rec", name=f"rec{t}")
            nc.vector.reciprocal(rec[:], ps_o[64:65, :, :])
            rec_b = ntmp.tile([64, 2, 512], F32, tag="recb", name=f"recb{t}")
            nc.gpsimd.partition_broadcast(rec_b[:], rec[:])
            nc.vector.tensor_mul(aall[0:64, t, :], ps_o[0:64, 0, :],
                                 rec_b[:, 0, :])
            nc.vector.tensor_mul(aall[64:128, t, :], ps_o[0:64, 1, :],
                                 rec_b[:, 1, :])

    for qc in range(NSC):
        aall_prev = aall
        aall = aall_pool.tile([P, NDT, 512], BF16, tag="aall", name=f"aall{qc}")
        for t in range(NDT):
            Kt = krot[t]
            for ktq in range(NST // NKQ):
                if qc == 0 and (ktq * NKQ) % 4 == 0:
                    sc = ktq * NKQ // 4
                    if t == 0:
                        # stream the remaining kT/vT chunks just ahead of use
                        if sc + 2 < NSC:
                            nc.gpsimd.dma_start(
                                out=ktc_a[:, sc + 2],
                                in_=kT[:, (sc + 2) * 512:(sc + 3) * 512]
                                .rearrange("(a p) c -> p a c", p=P))
                        if sc + 1 < NSC:
                            vload(sc + 1)
                    # prefetch the next K chunk / next tile's first pieces
                    if sc + 1 < NSC:
                        kproj_piece(t, sc + 1)
                    elif t + 1 < NDT:
                        qproj0_piece(t + 1)
                        kproj_piece(t + 1, 0)
                    if t == 0:
                        vproj_full(sc)
                ptt = pt_pool.tile([P, NKQ, 2, 512], BF16, tag="pt",
                                   name=f"pt{qc}_{t}_{ktq}")
                for j in range(NKQ):
                    kt = ktq * NKQ + j
                    ksl = slice(kt * P, (kt + 1) * P)
                    if pending is not None and j > 0:
                        # spread the pending chunk between the scores pairs
                        emit_attnv_part(pending[0], pending[1], pending[2],
                                        j - 1)
                    psS = psum_s.tile([P, 2, 512], F32, tag="ps",
                                      name=f"psS{qc}{t}{kt}")
                    tpA = (0, 0) if SCORES_PACK else None
                    tpB = (64, 0) if SCORES_PACK else None
                    nc.tensor.matmul(psS[:, 0, :], Kt[0:64, ksl],
                                     qr_cur[0:64, t, :], start=True, stop=True,
                                     tile_position=tpA)
                    nc.tensor.matmul(psS[:, 1, :], Kt[64:128, ksl],
                                     qr_cur[64:128, t, :], start=True, stop=True,
                                     tile_position=tpB)
                    if EXP_PATH == "direct":
                        nc.scalar.activation(ptt[:, j, :, :], psS[:],
                                             mybir.ActivationFunctionType.Exp,
                                             scale=0.125)
                    else:
                        sa = stg.tile([P, 2, 512], BF16, tag="sa", name="sa")
                        nc.scalar.mul(sa[:], psS[:], 0.125)
                        nc.scalar.activation(ptt[:, j, :, :], sa[:],
                                             mybir.ActivationFunctionType.Exp)
                if pending is not None:
                    # last part + the normalize tail of the pending chunk
                    emit_attnv(*pending, skip_parts=NKQ - 1)
                pending = (t, ktq, ptt, aall)
            # PE filler between d-tiles (keeps PE fed while ScalarE
            # drains), spread evenly: one next-chunk Q-projection piece and
            # one previous-chunk output s-tile per d-tile.
            if qc + 1 < NSC:
                if t == 0:
                    qr_next = qproj_load(qc + 1)
                qproj_piece(qc + 1, t)
            if qc >= 1:
                outproj_st(qc - 1, aall_prev, t)
        if qc + 1 < NSC:
            qr_cur = qr_next
    # drain the last attnv chunk + the final output projections
    if pending is not None:
        emit_attnv(*pending, skip_parts=0)
        pending = None
    for sti in range(4):
        outproj_st(NSC - 1, aall, sti)


# ---------------------------------------------------------------------------
# host side
# ---------------------------------------------------------------------------

_PROGRAM = None


def _get_program():
    global _PROGRAM
    if _PROGRAM is None:
        _PROGRAM = build_program()
    return _PROGRAM


def _np_bf16():
    import ml_dtypes

    return np.dtype(ml_dtypes.bfloat16)


def _perm_rows(hh):
    """Row permutation of Wq/Wk for one head-half.

    Per head h: [h evens (32) | h odds (32)], heads consecutive. Within a
    128-tile t: head 2t rows 0:64, head 2t+1 rows 64:128; RoPE pairs sit at
    partition distance 32 inside each head's 64 rows.
    """
    base = hh * HPC * D
    rows = []
    for h in range(HPC):
        a = base + h * D
        rows += [a + 2 * i for i in range(32)]
        rows += [a + 2 * i + 1 for i in range(32)]
    return np.array(rows, dtype=np.int64)


def _tables():
    inv_freq = 1.0 / (10000.0 ** (np.arange(0, D, 2, dtype=np.float32) / D))
    freqs = np.arange(S, dtype=np.float32)[:, None] * inv_freq[None, :]  # [S, 32]
    cos = np.cos(freqs).T.astype(np.float32)  # [32, S]
    sin = np.sin(freqs).T.astype(np.float32)
    C = np.tile(cos, (4, 1))  # [128, S]
    Ssig = np.concatenate([-sin, sin, -sin, sin], axis=0).astype(np.float32)
    dt = _np_bf16()
    return np.ascontiguousarray(C).astype(dt), np.ascontiguousarray(Ssig).astype(dt)


def prepare_inputs(query, key, value, Wq, Wk, Wv, Wo, bo):
    dt = _np_bf16()
    C, Ssig = _tables()
    xTs = {}
    for b in range(B):
        xTs[b] = tuple(
            np.ascontiguousarray(np.asarray(x[b], np.float32).T).astype(dt)
            for x in (query, key, value)
        )
    per_hh = {}
    for hh in range(2):
        perm = _perm_rows(hh)
        dsl = slice(hh * DPC, (hh + 1) * DPC)
        per_hh[hh] = {
            "wqT": np.ascontiguousarray(np.asarray(Wq, np.float32)[perm, :].T).astype(dt),
            "wkT": np.ascontiguousarray(np.asarray(Wk, np.float32)[perm, :].T).astype(dt),
            "wvT": np.ascontiguousarray(np.asarray(Wv, np.float32)[dsl, :].T).astype(dt),
            "woT": np.ascontiguousarray(np.asarray(Wo, np.float32)[:, dsl].T).astype(dt),
        }
    in_maps = []
    for c in range(8):
        b, hh = c // 2, c % 2
        qTb, kTb, vTb = xTs[b]
        m = {"qT": qTb, "kT": kTb, "vT": vTb, "ctab": C, "stab": Ssig}
        m.update(per_hh[hh])
        in_maps.append(m)
    return in_maps


def kernel(query, key, value, Wq, Wk, Wv, Wo, bo):
    nc = _get_program()
    in_maps = prepare_inputs(query, key, value, Wq, Wk, Wv, Wo, bo)
    res = run_bass_kernel_spmd(nc, in_maps, list(range(8)))
    bo = np.asarray(bo, np.float32)
    out = np.empty((B, S, E), np.float32)
    for b in range(B):
        out[b] = res.results[b * 2]["out"] + res.results[b * 2 + 1]["out"] + bo
    return out


if __name__ == "__main__":
    rng = np.random.default_rng(0)
    ins = {
        "query": rng.standard_normal((B, S, E)).astype(np.float32),
        "key": rng.standard_normal((B, S, E)).astype(np.float32),
        "value": rng.standard_normal((B, S, E)).astype(np.float32),
        "Wq": (rng.standard_normal((E, E)) * 0.02).astype(np.float32),
        "Wk": (rng.standard_normal((E, E)) * 0.02).astype(np.float32),
        "Wv": (rng.standard_normal((E, E)) * 0.02).astype(np.float32),
        "Wo": (rng.standard_normal((E, E)) * 0.02).astype(np.float32),
        "bo": np.zeros((E,), np.float32),
    }
    o = kernel(**ins)
    print("out", o.shape, o.dtype, float(np.abs(o).max()))



# revision 6
# speedup vs baseline: 1.0244x; 1.0244x over previous
"""Trainium2 Bass kernel for nn_MultiHeadAttention (B=4, S=2048, E=1024, H=16, D=64).

Sharding: 8 cores = 4 batches x 2 head-halves. Core c handles batch c//2 and
heads [ (c%2)*8, (c%2)*8+8 ). Each core computes its heads' attention and a
partial output projection; the host sums the two per-batch partials and adds bo.

v2 design (CoreSim cost model: matmul = out-free-cols * 0.4167ns, act =
free-elems * 0.8333ns + ~185ns/instr):
  - ScalarE exp (256 x [128,1024] activations = ~266us) is the floor; PE work
    is packed to ~253us so the act stream never starves.
  - attnv is q-major: out[q,d] = P^T-tile (stationary, 128 q cols) x V
    (moving, 64 cols) -> 64 cyc per matmul instead of 512 (d-major), i.e.
    attnv 262k -> 135k cycles. Softmax denominators come from 1-column
    ones-matmuls (1 cyc each) accumulated in a dedicated PSUM bank under a
    single long start/stop group per half (4 heads).
  - Q/K/V projections use fp8e4m3 DoubleRow with residual correction:
    w8@x8 + wr8@x8 + w8@xr8 (12 DR matmuls x 256 cyc = 3072 vs 4096 bf16).
    Host uploads w8/wr8 (x64 scale, descaled via the 1/64-baked rope tables
    and the V eviction scale) and x8/xr8. Error ~0.2% (vs 3.6% straight fp8).
  - Scores + exp unchanged (bf16, contract 64, head pairs row-tiled into the
    two banks of a [P,2,512] PSUM tile, one activation evicts both).
  - Normalize: per-half reciprocal of D [128,16] + per-(qt,head)
    tensor_scalar_mul evictions (PSUM f32 -> bf16 asb [q, hd]); PE
    transposes (via identity) rebuild aall [hd, q] for the (unchanged)
    bf16 output projection.
  - PSUM map (8 banks): scores 2x2, attnv 2x1 (4 heads x 2 qt per bank,
    shared accumulation group), D 1, proj/transpose 1.
"""

import os
import sys
import numpy as np

sys.path.insert(0, "/opt/trn_rl_repo")

from contextlib import ExitStack

import concourse.bacc as bacc
import concourse.tile as tile
from concourse import mybir
from concourse.bass_utils import run_bass_kernel_spmd

B, S, E = 4, 2048, 1024
H, D = 16, 64
HPC = 8          # heads per core
DPC = HPC * D    # 512 d-dims per core
P = 128
NSC = S // 512   # 4 s-chunks of 512
NST = S // 128   # 16 s-tiles of 128
NET = E // 128   # 8 e-tiles of 128
NDT = DPC // 128  # 4 d-tiles of 128 (= head pairs)
NKQ = 4          # kt tiles per pt chunk

F32 = mybir.dt.float32
BF16 = mybir.dt.bfloat16
FP8 = mybir.dt.float8e4
DRM = mybir.MatmulPerfMode.DoubleRow

WSCALE = 64.0    # fp8 weight scale; descaled via rope tables / evict muls

REPEAT = int(os.environ.get("KERNEL_REPEAT", "1"))
PT_BUFS = int(os.environ.get("PT_BUFS", "4"))


def build_program(repeat=None):
    global REPEAT
    if repeat is not None:
        REPEAT = repeat
    nc = bacc.Bacc("TRN2", target_bir_lowering=False, debug=False, num_devices=8)

    # paired fp8 inputs: [:, 0] = main, [:, 1] = residual
    q8 = nc.dram_tensor("q8", [2, E, S], FP8, kind="ExternalInput").ap()
    k8 = nc.dram_tensor("k8", [2, E, S], FP8, kind="ExternalInput").ap()
    v8 = nc.dram_tensor("v8", [2, E, S], FP8, kind="ExternalInput").ap()
    wq8 = nc.dram_tensor("wq8", [2, E, DPC], FP8, kind="ExternalInput").ap()
    wk8 = nc.dram_tensor("wk8", [2, E, DPC], FP8, kind="ExternalInput").ap()
    wv8 = nc.dram_tensor("wv8", [2, E, DPC], FP8, kind="ExternalInput").ap()
    woT = nc.dram_tensor("woT", [DPC, E], BF16, kind="ExternalInput").ap()
    ctab = nc.dram_tensor("ctab", [P, S], BF16, kind="ExternalInput").ap()
    stab = nc.dram_tensor("stab", [P, S], BF16, kind="ExternalInput").ap()
    ident = nc.dram_tensor("ident", [P, P], BF16, kind="ExternalInput").ap()
    out = nc.dram_tensor("out", [S, E], F32, kind="ExternalOutput").ap()

    with tile.TileContext(nc) as tc:
        with ExitStack() as ctx:
            body(ctx, tc, nc, q8, k8, v8, wq8, wk8, wv8, woT, ctab, stab,
                 ident, out)
    nc.compile()
    return nc


def body(ctx, tc, nc, q8, k8, v8, wq8, wk8, wv8, woT, ctab, stab, ident, out):
    consts = ctx.enter_context(tc.tile_pool(name="consts", bufs=1))
    c_sb = consts.tile([P, S], BF16, tag="ctab")
    s_sb = consts.tile([P, S], BF16, tag="stab")
    id_sb = consts.tile([P, P], BF16, tag="ident")
    ones = consts.tile([P, 1], BF16, tag="ones")
    nc.vector.memset(ones[:], 1.0)

    wpool = ctx.enter_context(tc.tile_pool(name="wpool", bufs=1))
    wopool = ctx.enter_context(tc.tile_pool(name="wopool", bufs=1))
    ktpool = ctx.enter_context(tc.tile_pool(name="ktpool", bufs=1))
    xq = ctx.enter_context(tc.tile_pool(name="xq", bufs=2))
    xv = ctx.enter_context(tc.tile_pool(name="xv", bufs=2))

    krot_pool = ctx.enter_context(tc.tile_pool(name="krot", bufs=4))
    qrot_pool = ctx.enter_context(tc.tile_pool(name="qrot", bufs=2))
    vreg_pool = ctx.enter_context(tc.tile_pool(name="vreg", bufs=1))
    aall_pool = ctx.enter_context(tc.tile_pool(name="aall", bufs=2))
    asb_pool = ctx.enter_context(tc.tile_pool(name="asb", bufs=2))
    rec_pool = ctx.enter_context(tc.tile_pool(name="rec", bufs=2))
    pt_pool = ctx.enter_context(tc.tile_pool(name="pt", bufs=PT_BUFS))

    rtmp = ctx.enter_context(tc.tile_pool(name="rtmp", bufs=2))
    opool = ctx.enter_context(tc.tile_pool(name="opool", bufs=2))

    psum_s = ctx.enter_context(tc.tile_pool(name="psum_s", bufs=2, space="PSUM"))
    psum_av = ctx.enter_context(tc.tile_pool(name="psum_av", bufs=2, space="PSUM"))
    psum_d = ctx.enter_context(tc.tile_pool(name="psum_d", bufs=1, space="PSUM"))
    psum_p = ctx.enter_context(tc.tile_pool(name="psum_p", bufs=1, space="PSUM"))

    # weights + kT resident for the whole kernel, loaded with combined strided
    # DMAs split between the SP and GpSimd issue queues, ordered by first
    # consumption (wk + k(sc0) + wq + q(sc0) + tables gate the first exp).
    def load_pair(pool, tag, dram, cols, n=NET, eng=None):
        t_ = pool.tile([P, 2, n, cols], FP8, tag=tag, name=tag)
        e = eng or nc.sync
        e.dma_start(out=t_[:, 0], in_=dram[0].rearrange("(a p) c -> p a c", p=P))
        e.dma_start(out=t_[:, 1], in_=dram[1].rearrange("(a p) c -> p a c", p=P))
        return t_

    wk_a = load_pair(wpool, "wk", wk8, DPC)
    ktc_a = ktpool.tile([P, 2, NSC, NET, 512], FP8, tag="kt", name="ktc")
    for r in range(2):
        nc.gpsimd.dma_start(
            out=ktc_a[:, r, 0],
            in_=k8[r, :, 0:512].rearrange("(a p) c -> p a c", p=P))
    wq_a = load_pair(wpool, "wq", wq8, DPC, eng=nc.gpsimd)
    xq0 = xq.tile([P, 2, NET, 512], FP8, tag="xqp", name="x_q0")
    for r in range(2):
        nc.sync.dma_start(out=xq0[:, r],
                          in_=q8[r, :, 0:512].rearrange("(a p) c -> p a c", p=P))
    nc.sync.dma_start(out=c_sb[:, 0:512], in_=ctab[:, 0:512])
    nc.gpsimd.dma_start(out=s_sb[:, 0:512], in_=stab[:, 0:512])
    nc.sync.dma_start(out=id_sb[:], in_=ident)
    nc.sync.dma_start(out=c_sb[:, 512:], in_=ctab[:, 512:])
    nc.gpsimd.dma_start(out=s_sb[:, 512:], in_=stab[:, 512:])
    for r in range(2):
        nc.sync.dma_start(
            out=ktc_a[:, r, 1],
            in_=k8[r, :, 512:1024].rearrange("(a p) c -> p a c", p=P))
    wv_a = load_pair(wpool, "wv", wv8, DPC, eng=nc.gpsimd)
    vt0 = xv.tile([P, 2, NET, 512], FP8, tag="xv", name="x_v0")
    for r in range(2):
        nc.sync.dma_start(out=vt0[:, r],
                          in_=v8[r, :, 0:512].rearrange("(a p) c -> p a c", p=P))
    wo_a = wopool.tile([P, NDT, E], BF16, tag="wo", name="wo")
    nc.gpsimd.dma_start(out=wo_a[:],
                        in_=woT.rearrange("(a p) c -> p a c", p=P))
    wo_sb = [wo_a[:, t, :] for t in range(NDT)]

    pools = (c_sb, s_sb, id_sb, ones, wk_a, wq_a, wv_a, wo_sb, ktc_a,
             xq, xv, xq0, vt0,
             krot_pool, qrot_pool, vreg_pool, aall_pool, asb_pool, rec_pool,
             pt_pool, rtmp, opool, psum_s, psum_av, psum_d, psum_p)
    for rep in range(REPEAT):
        one_pass(tc, nc, q8, k8, v8, out, *pools)


def one_pass(tc, nc, q8, k8, v8, out,
             c_sb, s_sb, id_sb, ones, wk_a, wq_a, wv_a, wo_sb, ktc_a,
             xq, xv, xq0, vt0,
             krot_pool, qrot_pool, vreg_pool, aall_pool, asb_pool, rec_pool,
             pt_pool, rtmp, opool, psum_s, psum_av, psum_d, psum_p):

    def rope_evict(ps, dst, ssl):
        """ps [P,512] f32 PSUM (x WSCALE) -> dst [P,512] bf16 rotated.

        Row layout per 64 rows (one head): [32 evens | 32 odds]; RoPE pairs
        sit at partition distance 32, so the swap is between 32-blocks
        (0<->1, 2<->3). s_sb rows carry the sign: [-sin, sin, -sin, sin].
        Tables are pre-scaled by 1/WSCALE so the result is true-scale.
        """
        st_bf = rtmp.tile([P, 512], BF16, tag="st", name="st_bf")
        nc.vector.tensor_copy(st_bf[:], ps[:])
        xsw = rtmp.tile([P, 512], BF16, tag="xsw", name="xsw")
        for blk in range(4):
            sb = blk ^ 1
            nc.vector.tensor_copy(xsw[blk * 32:(blk + 1) * 32, :],
                                  st_bf[sb * 32:(sb + 1) * 32, :])
        nc.vector.tensor_mul(xsw[:], xsw[:], s_sb[:, ssl])
        t2 = rtmp.tile([P, 512], BF16, tag="t2", name="t2")
        nc.vector.tensor_mul(t2[:], st_bf[:], c_sb[:, ssl])
        nc.vector.tensor_add(dst, t2[:], xsw[:])

    def proj_chunk(w_a, x_a, t, nm):
        # fp8 DoubleRow residual projection: w8@x8 + wr8@x8 + w8@xr8,
        # 12 DR matmuls (et-pairs) accumulating into one [P,512] group.
        ps = psum_p.tile([P, 512], F32, tag="pp", name=f"pp_{nm}")
        tsl = slice(t * P, (t + 1) * P)
        terms = ((0, 0), (1, 0), (0, 1))
        for ti, (wr, xr) in enumerate(terms):
            for p8 in range(NET // 2):
                esl = slice(2 * p8, 2 * p8 + 2)
                nc.tensor.matmul(
                    ps[:], w_a[:, wr, esl, tsl], x_a[:, xr, esl, :],
                    start=(ti == 0 and p8 == 0),
                    stop=(ti == 2 and p8 == NET // 2 - 1),
                    perf_mode=DRM,
                )
        return ps

    krot = [krot_pool.tile([P, S], BF16, tag="krot", name=f"krot{i}")
            for i in range(NDT)]
    vreg = vreg_pool.tile([P, NST, HPC * 64], BF16, tag="vreg")
    vreg_v = vreg.rearrange("p st (h dd) -> p st h dd", h=HPC)

    def kproj_piece(t, sc):
        ssl = slice(sc * 512, (sc + 1) * 512)
        ps = proj_chunk(wk_a, ktc_a[:, :, sc], t, f"k{t}{sc}")
        rope_evict(ps, krot[t][:, ssl], ssl)

    qp_state = {}

    def qproj_load(qc):
        qsl = slice(qc * 512, (qc + 1) * 512)
        xq_a = xq.tile([P, 2, NET, 512], FP8, tag="xqp", name=f"x_q{qc}")
        for r in range(2):
            nc.gpsimd.dma_start(
                out=xq_a[:, r],
                in_=q8[r, :, qsl].rearrange("(a p) c -> p a c", p=P))
        qr = qrot_pool.tile([P, NDT, 512], BF16, tag="qrot", name=f"qr{qc}")
        qp_state[qc] = (xq_a, qr)
        return qr

    def qproj_piece(qc, t):
        xq_a, qr = qp_state[qc]
        qsl = slice(qc * 512, (qc + 1) * 512)
        ps = proj_chunk(wq_a, xq_a, t, f"q{qc}{t}")
        rope_evict(ps, qr[:, t, :], qsl)

    vload_pending = {}

    def vload(sc):
        # prefetch the vT chunk one scores-chunk ahead of its vproj
        xv_a = xv.tile([P, 2, NET, 512], FP8, tag="xv", name=f"x_v{sc}")
        for r in range(2):
            nc.gpsimd.dma_start(
                out=xv_a[:, r],
                in_=v8[r, :, sc * 512:(sc + 1) * 512].rearrange(
                    "(a p) c -> p a c", p=P))
        vload_pending[sc] = xv_a

    def vproj_full(sc):
        # V for all 8 heads over the 4 s-tiles of chunk sc; DR residual with
        # x as the stationary operand (out [s, hd]); evict with 1/WSCALE.
        xv_a = vt0 if sc == 0 else vload_pending.pop(sc)
        for sti in range(4):
            st = sc * 4 + sti
            ps = psum_p.tile([P, 512], F32, tag="pp", name=f"pp_v{sc}{st}")
            ssl = slice(sti * P, (sti + 1) * P)
            terms = ((0, 0), (1, 0), (0, 1))
            for ti, (xr, wr) in enumerate(terms):
                for p8 in range(NET // 2):
                    esl = slice(2 * p8, 2 * p8 + 2)
                    nc.tensor.matmul(
                        ps[:], xv_a[:, xr, esl, ssl], wv_a[:, wr, esl, :],
                        start=(ti == 0 and p8 == 0),
                        stop=(ti == 2 and p8 == NET // 2 - 1),
                        perf_mode=DRM,
                    )
            nc.vector.tensor_scalar_mul(
                vreg_v[:, st, :, 0:64],
                ps[:].rearrange("p (h d) -> p h d", h=HPC),
                1.0 / WSCALE,
            )

    def outproj_st(qc, aall, sti):
        # output projection for one s-tile of chunk qc (bf16, unchanged)
        st = qc * 4 + sti
        osb = opool.tile([P, E], F32, tag="osb", name=f"osb{st}")
        for ec in range(2):
            esl = slice(ec * 512, (ec + 1) * 512)
            ps_f = psum_p.tile([P, 512], F32, tag="pp", name=f"pp_f{st}{ec}")
            for t in range(NDT):
                nc.tensor.matmul(ps_f[:],
                                 aall[:, t, sti * P:(sti + 1) * P],
                                 wo_sb[t][:, esl],
                                 start=(t == 0), stop=(t == NDT - 1))
            nc.vector.tensor_copy(osb[:, esl], ps_f[:])
        nc.sync.dma_start(out=out[st * P:(st + 1) * P, :], in_=osb[:])

    # ---------------- attention, software-pipelined per q-chunk -------------
    qr0 = qrot_pool.tile([P, NDT, 512], BF16, tag="qrot", name="qr0")

    def qproj0_piece(t):
        ps = proj_chunk(wq_a, xq0, t, f"q0{t}")
        rope_evict(ps, qr0[:, t, :], slice(0, 512))

    kproj_piece(0, 0)
    qproj0_piece(0)

    qr_cur = qr0
    aall_prev = None
    aall = None
    asb_prev = None
    asb = None
    pending = None  # (t, ktq, ptt, asb) attnv chunk awaiting emission
    av_state = {}   # half H -> (av_tiles[2], d_tile)
    half_ctr = [0]

    def start_half(H):
        n = half_ctr[0]
        half_ctr[0] += 1
        avs = [psum_av.tile([P, 2, 4, 64], F32, tag="av", name=f"av{n}_{i}")
               for i in range(2)]
        d_t = psum_d.tile([P, 16], F32, tag="d", name=f"d{n}")
        av_state[H] = (avs, d_t)

    def emit_attnv_part(t, ktq, ptt, j):
        # q-major attnv + denominator for one kt of a pending chunk
        H = t // 2
        if t % 2 == 0 and ktq == 0 and j == 0:
            start_half(H)
        avs, d_t = av_state[H]
        first = (t % 2 == 0 and ktq == 0 and j == 0)
        last = (t % 2 == 1 and ktq == NST // NKQ - 1 and j == NKQ - 1)
        kt = ktq * NKQ + j
        for qt in range(4):
            qsl = slice(qt * P, (qt + 1) * P)
            av = avs[qt // 2]
            for gl in range(2):
                hl = 2 * (t % 2) + gl
                hg = 2 * t + gl
                nc.tensor.matmul(
                    av[:, qt % 2, hl, :],
                    ptt[:, j, gl, qsl], vreg_v[:, kt, hg, :],
                    start=(first and qt % 2 == 0 and gl == 0),
                    stop=(last and qt % 2 == 1 and gl == 1),
                )
                nc.tensor.matmul(
                    d_t[:, qt * 4 + hl:qt * 4 + hl + 1],
                    ptt[:, j, gl, qsl], ones[:, 0:1],
                    start=(first and qt == 0 and gl == 0),
                    stop=(last and qt == 3 and gl == 1),
                )

    def finish_half(H, asb_t):
        # reciprocal of the 16 denominators, then evict+normalize the 16
        # [128,64] blocks of the half into asb (bf16, [q, hd])
        avs, d_t = av_state.pop(H)
        rec = rec_pool.tile([P, 16], F32, tag="rec",
                            name=f"rec{half_ctr[0]}")
        nc.vector.reciprocal(rec[:], d_t[:])
        for qt in range(4):
            av = avs[qt // 2]
            for hl in range(4):
                hg = 4 * H + hl
                nc.vector.tensor_scalar_mul(
                    asb_t[:, qt, hg * 64:(hg + 1) * 64],
                    av[:, qt % 2, hl, :],
                    rec[:, qt * 4 + hl:qt * 4 + hl + 1],
                )

    def emit_attnv(t, ktq, ptt, skip_parts=0):
        for j in range(skip_parts, NKQ):
            emit_attnv_part(t, ktq, ptt, j)

    def transpose_qt(asb_t, qt, aall_t):
        # asb [q, hd] -> aall [hd, q] via PE transpose, 4 [128,128] pieces
        for tt in range(NDT):
            pst = psum_p.tile([P, P], BF16, tag="pp", name=f"tp{qt}{tt}")
            nc.tensor.transpose(pst[:], asb_t[:, qt, tt * P:(tt + 1) * P],
                                id_sb[:])
            nc.vector.tensor_copy(aall_t[:, tt, qt * P:(qt + 1) * P], pst[:])

    for qc in range(NSC):
        aall_prev = aall
        asb_prev = asb
        aall = aall_pool.tile([P, NDT, 512], BF16, tag="aall", name=f"aall{qc}")
        asb = asb_pool.tile([P, 4, 512], BF16, tag="asb", name=f"asb{qc}")
        for t in range(NDT):
            Kt = krot[t]
            for ktq in range(NST // NKQ):
                if qc == 0 and (ktq * NKQ) % 4 == 0:
                    sc = ktq * NKQ // 4
                    if t == 0:
                        # stream the remaining kT/vT chunks just ahead of use
                        if sc + 2 < NSC:
                            for r in range(2):
                                nc.gpsimd.dma_start(
                                    out=ktc_a[:, r, sc + 2],
                                    in_=k8[r, :, (sc + 2) * 512:(sc + 3) * 512]
                                    .rearrange("(a p) c -> p a c", p=P))
                        if sc + 1 < NSC:
                            vload(sc + 1)
                    # prefetch the next K chunk / next tile's first pieces
                    if sc + 1 < NSC:
                        kproj_piece(t, sc + 1)
                    elif t + 1 < NDT:
                        qproj0_piece(t + 1)
                        kproj_piece(t + 1, 0)
                    if t == 0:
                        vproj_full(sc)
                ptt = pt_pool.tile([P, NKQ, 2, 512], BF16, tag="pt",
                                   name=f"pt{qc}_{t}_{ktq}")
                for j in range(NKQ):
                    kt = ktq * NKQ + j
                    ksl = slice(kt * P, (kt + 1) * P)
                    if pending is not None and j > 0:
                        # spread the pending chunk between the scores pairs
                        emit_attnv_part(pending[0], pending[1], pending[2],
                                        j - 1)
                    psS = psum_s.tile([P, 2, 512], F32, tag="ps",
                                      name=f"psS{qc}{t}{kt}")
                    nc.tensor.matmul(psS[:, 0, :], Kt[0:64, ksl],
                                     qr_cur[0:64, t, :], start=True, stop=True,
                                     tile_position=(0, 0))
                    nc.tensor.matmul(psS[:, 1, :], Kt[64:128, ksl],
                                     qr_cur[64:128, t, :], start=True, stop=True,
                                     tile_position=(64, 0))
                    nc.scalar.activation(ptt[:, j, :, :], psS[:],
                                         mybir.ActivationFunctionType.Exp,
                                         scale=0.125)
                if pending is not None:
                    # last part of the pending chunk (+ half finish if due)
                    pt_, ktq_, ptt_, asb_ = pending
                    emit_attnv_part(pt_, ktq_, ptt_, NKQ - 1)
                    if pt_ % 2 == 1 and ktq_ == NST // NKQ - 1:
                        finish_half(pt_ // 2, asb_)
                pending = (t, ktq, ptt, asb)
            # PE filler between d-tiles: next-chunk Q-projection piece,
            # previous-chunk transposes + output s-tile.
            if qc + 1 < NSC:
                if t == 0:
                    qr_next = qproj_load(qc + 1)
                qproj_piece(qc + 1, t)
            if qc >= 1:
                transpose_qt(asb_prev, t, aall_prev)
                outproj_st(qc - 1, aall_prev, t)
        if qc + 1 < NSC:
            qr_cur = qr_next
    # drain the last attnv chunk, finish the last half, final transposes +
    # output projections
    if pending is not None:
        pt_, ktq_, ptt_, asb_ = pending
        emit_attnv(pt_, ktq_, ptt_, skip_parts=0)
        finish_half(pt_ // 2, asb_)
        pending = None
    for sti in range(4):
        transpose_qt(asb, sti, aall)
        outproj_st(NSC - 1, aall, sti)


# ---------------------------------------------------------------------------
# host side
# ---------------------------------------------------------------------------

_PROGRAM = None


def _get_program():
    global _PROGRAM
    if _PROGRAM is None:
        _PROGRAM = build_program()
    return _PROGRAM


def _np_bf16():
    import ml_dtypes

    return np.dtype(ml_dtypes.bfloat16)


def _np_fp8():
    import ml_dtypes

    return np.dtype(ml_dtypes.float8_e4m3)


def _fp8_pair(a):
    """a (f32) -> stacked [2, ...] fp8e4m3 (main, residual)."""
    dt8 = _np_fp8()
    a8 = a.astype(dt8)
    r8 = (a - a8.astype(np.float32)).astype(dt8)
    return np.ascontiguousarray(np.stack([a8, r8], axis=0))


def _perm_rows(hh):
    """Row permutation of Wq/Wk for one head-half.

    Per head h: [h evens (32) | h odds (32)], heads consecutive. Within a
    128-tile t: head 2t rows 0:64, head 2t+1 rows 64:128; RoPE pairs sit at
    partition distance 32 inside each head's 64 rows.
    """
    base = hh * HPC * D
    rows = []
    for h in range(HPC):
        a = base + h * D
        rows += [a + 2 * i for i in range(32)]
        rows += [a + 2 * i + 1 for i in range(32)]
    return np.array(rows, dtype=np.int64)


def _tables():
    inv_freq = 1.0 / (10000.0 ** (np.arange(0, D, 2, dtype=np.float32) / D))
    freqs = np.arange(S, dtype=np.float32)[:, None] * inv_freq[None, :]  # [S, 32]
    cos = np.cos(freqs).T.astype(np.float32) / WSCALE  # [32, S]
    sin = np.sin(freqs).T.astype(np.float32) / WSCALE
    C = np.tile(cos, (4, 1))  # [128, S]
    Ssig = np.concatenate([-sin, sin, -sin, sin], axis=0).astype(np.float32)
    dt = _np_bf16()
    return np.ascontiguousarray(C).astype(dt), np.ascontiguousarray(Ssig).astype(dt)


def prepare_inputs(query, key, value, Wq, Wk, Wv, Wo, bo):
    dt = _np_bf16()
    C, Ssig = _tables()
    ident = np.eye(P, dtype=np.float32).astype(dt)
    xTs = {}
    for b in range(B):
        xTs[b] = tuple(
            _fp8_pair(np.ascontiguousarray(np.asarray(x[b], np.float32).T))
            for x in (query, key, value)
        )
    per_hh = {}
    for hh in range(2):
        perm = _perm_rows(hh)
        dsl = slice(hh * DPC, (hh + 1) * DPC)
        per_hh[hh] = {
            "wq8": _fp8_pair(
                np.ascontiguousarray(np.asarray(Wq, np.float32)[perm, :].T) * WSCALE),
            "wk8": _fp8_pair(
                np.ascontiguousarray(np.asarray(Wk, np.float32)[perm, :].T) * WSCALE),
            "wv8": _fp8_pair(
                np.ascontiguousarray(np.asarray(Wv, np.float32)[dsl, :].T) * WSCALE),
            "woT": np.ascontiguousarray(
                np.asarray(Wo, np.float32)[:, dsl].T).astype(dt),
        }
    in_maps = []
    for c in range(8):
        b, hh = c // 2, c % 2
        q8b, k8b, v8b = xTs[b]
        m = {"q8": q8b, "k8": k8b, "v8": v8b, "ctab": C, "stab": Ssig,
             "ident": ident}
        m.update(per_hh[hh])
        in_maps.append(m)
    return in_maps


def kernel(query, key, value, Wq, Wk, Wv, Wo, bo):
    nc = _get_program()
    in_maps = prepare_inputs(query, key, value, Wq, Wk, Wv, Wo, bo)
    res = run_bass_kernel_spmd(nc, in_maps, list(range(8)))
    bo = np.asarray(bo, np.float32)
    out = np.empty((B, S, E), np.float32)
    for b in range(B):
        out[b] = res.results[b * 2]["out"] + res.results[b * 2 + 1]["out"] + bo
    return out


if __name__ == "__main__":
    rng = np.random.default_rng(0)
    ins = {
        "query": rng.standard_normal((B, S, E)).astype(np.float32),
        "key": rng.standard_normal((B, S, E)).astype(np.float32),
        "value": rng.standard_normal((B, S, E)).astype(np.float32),
        "Wq": (rng.standard_normal((E, E)) * 0.02).astype(np.float32),
        "Wk": (rng.standard_normal((E, E)) * 0.02).astype(np.float32),
        "Wv": (rng.standard_normal((E, E)) * 0.02).astype(np.float32),
        "Wo": (rng.standard_normal((E, E)) * 0.02).astype(np.float32),
        "bo": np.zeros((E,), np.float32),
    }
    o = kernel(**ins)
    print("out", o.shape, o.dtype, float(np.abs(o).max()))


# revision 9
# speedup vs baseline: 1.0483x; 1.0233x over previous
"""Trainium2 Bass kernel for nn_MultiHeadAttention (B=4, S=2048, E=1024, H=16, D=64).

Sharding: 8 cores = 4 batches x 2 head-halves. Core c handles batch c//2 and
heads [ (c%2)*8, (c%2)*8+8 ). Each core computes its heads' attention and a
partial output projection; the host sums the two per-batch partials and adds bo.

v2 design (CoreSim cost model: matmul = out-free-cols * 0.4167ns, act =
free-elems * 0.8333ns + ~185ns/instr):
  - ScalarE exp (256 x [128,1024] activations = ~266us) is the floor; PE work
    is packed to ~253us so the act stream never starves.
  - attnv is q-major: out[q,d] = P^T-tile (stationary, 128 q cols) x V
    (moving, 64 cols) -> 64 cyc per matmul instead of 512 (d-major), i.e.
    attnv 262k -> 135k cycles. Softmax denominators come from 1-column
    ones-matmuls (1 cyc each) accumulated in a dedicated PSUM bank under a
    single long start/stop group per half (4 heads).
  - Q/K/V projections use fp8e4m3 DoubleRow with residual correction:
    w8@x8 + wr8@x8 + w8@xr8 (12 DR matmuls x 256 cyc = 3072 vs 4096 bf16).
    Host uploads w8/wr8 (x64 scale, descaled via the 1/64-baked rope tables
    and the V eviction scale) and x8/xr8. Error ~0.2% (vs 3.6% straight fp8).
  - Scores + exp unchanged (bf16, contract 64, head pairs row-tiled into the
    two banks of a [P,2,512] PSUM tile, one activation evicts both).
  - Normalize: per-half reciprocal of D [128,16] + per-(qt,head)
    tensor_scalar_mul evictions (PSUM f32 -> bf16 asb [q, hd]); PE
    transposes (via identity) rebuild aall [hd, q] for the (unchanged)
    bf16 output projection.
  - PSUM map (8 banks): scores 2x2, attnv 2x1 (4 heads x 2 qt per bank,
    shared accumulation group), D 1, proj/transpose 1.
"""

import os
import sys
import numpy as np

sys.path.insert(0, "/opt/trn_rl_repo")

from contextlib import ExitStack

import concourse.bacc as bacc
import concourse.tile as tile
from concourse import mybir
from concourse.bass_utils import run_bass_kernel_spmd

B, S, E = 4, 2048, 1024
H, D = 16, 64
HPC = 8          # heads per core
DPC = HPC * D    # 512 d-dims per core
P = 128
NSC = S // 512   # 4 s-chunks of 512
NST = S // 128   # 16 s-tiles of 128
NET = E // 128   # 8 e-tiles of 128
NDT = DPC // 128  # 4 d-tiles of 128 (= head pairs)
NKQ = 4          # kt tiles per pt chunk

F32 = mybir.dt.float32
BF16 = mybir.dt.bfloat16
FP8 = mybir.dt.float8e4
DRM = mybir.MatmulPerfMode.DoubleRow

WSCALE = 64.0    # fp8 weight scale; descaled via rope tables / evict muls

REPEAT = int(os.environ.get("KERNEL_REPEAT", "1"))
PT_BUFS = int(os.environ.get("PT_BUFS", "4"))


def build_program(repeat=None):
    global REPEAT
    if repeat is not None:
        REPEAT = repeat
    nc = bacc.Bacc("TRN2", target_bir_lowering=False, debug=False, num_devices=8)

    # paired fp8 inputs: [:, 0] = main, [:, 1] = residual
    q8 = nc.dram_tensor("q8", [2, E, S], FP8, kind="ExternalInput").ap()
    k8 = nc.dram_tensor("k8", [2, E, S], FP8, kind="ExternalInput").ap()
    v8 = nc.dram_tensor("v8", [2, E, S], FP8, kind="ExternalInput").ap()
    wq8 = nc.dram_tensor("wq8", [2, E, DPC], FP8, kind="ExternalInput").ap()
    wk8 = nc.dram_tensor("wk8", [2, E, DPC], FP8, kind="ExternalInput").ap()
    wv8 = nc.dram_tensor("wv8", [2, E, DPC], FP8, kind="ExternalInput").ap()
    woT = nc.dram_tensor("woT", [DPC, E], BF16, kind="ExternalInput").ap()
    ctab = nc.dram_tensor("ctab", [P, S], BF16, kind="ExternalInput").ap()
    stab = nc.dram_tensor("stab", [P, S], BF16, kind="ExternalInput").ap()
    ident = nc.dram_tensor("ident", [P, P], BF16, kind="ExternalInput").ap()
    out = nc.dram_tensor("out", [S, E], F32, kind="ExternalOutput").ap()

    with tile.TileContext(nc) as tc:
        with ExitStack() as ctx:
            body(ctx, tc, nc, q8, k8, v8, wq8, wk8, wv8, woT, ctab, stab,
                 ident, out)
    nc.compile()
    return nc


def body(ctx, tc, nc, q8, k8, v8, wq8, wk8, wv8, woT, ctab, stab, ident, out):
    consts = ctx.enter_context(tc.tile_pool(name="consts", bufs=1))
    c_sb = consts.tile([P, S], BF16, tag="ctab")
    s_sb = consts.tile([P, S], BF16, tag="stab")
    id_sb = consts.tile([P, P], BF16, tag="ident")
    ones = consts.tile([P, 1], BF16, tag="ones")
    nc.vector.memset(ones[:], 1.0)

    wpool = ctx.enter_context(tc.tile_pool(name="wpool", bufs=1))
    wopool = ctx.enter_context(tc.tile_pool(name="wopool", bufs=1))
    ktpool = ctx.enter_context(tc.tile_pool(name="ktpool", bufs=1))
    xq = ctx.enter_context(tc.tile_pool(name="xq", bufs=2))
    xv = ctx.enter_context(tc.tile_pool(name="xv", bufs=2))

    krot_pool = ctx.enter_context(tc.tile_pool(name="krot", bufs=4))
    qrot_pool = ctx.enter_context(tc.tile_pool(name="qrot", bufs=2))
    vreg_pool = ctx.enter_context(tc.tile_pool(name="vreg", bufs=1))
    aall_pool = ctx.enter_context(tc.tile_pool(name="aall", bufs=2))
    asb_pool = ctx.enter_context(tc.tile_pool(name="asb", bufs=2))
    rec_pool = ctx.enter_context(tc.tile_pool(name="rec", bufs=2))
    pt_pool = ctx.enter_context(tc.tile_pool(name="pt", bufs=PT_BUFS))

    rtmp = ctx.enter_context(tc.tile_pool(name="rtmp", bufs=2))
    opool = ctx.enter_context(tc.tile_pool(name="opool", bufs=2))

    psum_s = ctx.enter_context(tc.tile_pool(name="psum_s", bufs=2, space="PSUM"))
    psum_av = ctx.enter_context(tc.tile_pool(name="psum_av", bufs=2, space="PSUM"))
    psum_d = ctx.enter_context(tc.tile_pool(name="psum_d", bufs=1, space="PSUM"))
    psum_p = ctx.enter_context(tc.tile_pool(name="psum_p", bufs=1, space="PSUM"))

    # weights + kT resident for the whole kernel, loaded with combined strided
    # DMAs split between the SP and GpSimd issue queues, ordered by first
    # consumption (wk + k(sc0) + wq + q(sc0) + tables gate the first exp).
    def load_pair(pool, tag, dram, cols, n=NET, eng=None):
        t_ = pool.tile([P, 2, n, cols], FP8, tag=tag, name=tag)
        e = eng or nc.sync
        e.dma_start(out=t_[:, 0], in_=dram[0].rearrange("(a p) c -> p a c", p=P))
        e.dma_start(out=t_[:, 1], in_=dram[1].rearrange("(a p) c -> p a c", p=P))
        return t_

    # main-kind tensors first (they gate the first scores/exp); residuals are
    # only needed one DR term later.
    wk_a = wpool.tile([P, 2, NET, DPC], FP8, tag="wk", name="wk")
    ktc_a = ktpool.tile([P, 2, NSC, NET, 512], FP8, tag="kt", name="ktc")
    wq_a = wpool.tile([P, 2, NET, DPC], FP8, tag="wq", name="wq")
    xq0 = xq.tile([P, 2, NET, 512], FP8, tag="xqp", name="x_q0")
    nc.sync.dma_start(out=wk_a[:, 0],
                      in_=wk8[0].rearrange("(a p) c -> p a c", p=P))
    nc.gpsimd.dma_start(out=ktc_a[:, 0, 0],
                        in_=k8[0, :, 0:512].rearrange("(a p) c -> p a c", p=P))
    nc.sync.dma_start(out=c_sb[:, 0:512], in_=ctab[:, 0:512])
    nc.gpsimd.dma_start(out=s_sb[:, 0:512], in_=stab[:, 0:512])
    nc.gpsimd.dma_start(out=wq_a[:, 0],
                        in_=wq8[0].rearrange("(a p) c -> p a c", p=P))
    nc.sync.dma_start(out=xq0[:, 0],
                      in_=q8[0, :, 0:512].rearrange("(a p) c -> p a c", p=P))
    nc.sync.dma_start(out=wk_a[:, 1],
                      in_=wk8[1].rearrange("(a p) c -> p a c", p=P))
    nc.gpsimd.dma_start(out=ktc_a[:, 1, 0],
                        in_=k8[1, :, 0:512].rearrange("(a p) c -> p a c", p=P))
    nc.gpsimd.dma_start(out=wq_a[:, 1],
                        in_=wq8[1].rearrange("(a p) c -> p a c", p=P))
    nc.sync.dma_start(out=xq0[:, 1],
                      in_=q8[1, :, 0:512].rearrange("(a p) c -> p a c", p=P))
    nc.sync.dma_start(out=id_sb[:], in_=ident)
    nc.sync.dma_start(out=c_sb[:, 512:], in_=ctab[:, 512:])
    nc.gpsimd.dma_start(out=s_sb[:, 512:], in_=stab[:, 512:])
    for r in range(2):
        nc.sync.dma_start(
            out=ktc_a[:, r, 1],
            in_=k8[r, :, 512:1024].rearrange("(a p) c -> p a c", p=P))
    wv_a = load_pair(wpool, "wv", wv8, DPC, eng=nc.gpsimd)
    vt0 = xv.tile([P, 2, NET, 512], FP8, tag="xv", name="x_v0")
    for r in range(2):
        nc.sync.dma_start(out=vt0[:, r],
                          in_=v8[r, :, 0:512].rearrange("(a p) c -> p a c", p=P))
    wo_a = wopool.tile([P, NDT, E], BF16, tag="wo", name="wo")
    nc.gpsimd.dma_start(out=wo_a[:],
                        in_=woT.rearrange("(a p) c -> p a c", p=P))
    wo_sb = [wo_a[:, t, :] for t in range(NDT)]

    pools = (c_sb, s_sb, id_sb, ones, wk_a, wq_a, wv_a, wo_sb, ktc_a,
             xq, xv, xq0, vt0,
             krot_pool, qrot_pool, vreg_pool, aall_pool, asb_pool, rec_pool,
             pt_pool, rtmp, opool, psum_s, psum_av, psum_d, psum_p)
    for rep in range(REPEAT):
        one_pass(tc, nc, q8, k8, v8, out, *pools)


def one_pass(tc, nc, q8, k8, v8, out,
             c_sb, s_sb, id_sb, ones, wk_a, wq_a, wv_a, wo_sb, ktc_a,
             xq, xv, xq0, vt0,
             krot_pool, qrot_pool, vreg_pool, aall_pool, asb_pool, rec_pool,
             pt_pool, rtmp, opool, psum_s, psum_av, psum_d, psum_p):

    def rope_evict(ps, dst, ssl):
        """ps [P,512] f32 PSUM (x WSCALE) -> dst [P,512] bf16 rotated.

        Row layout per 64 rows (one head): [32 evens | 32 odds]; RoPE pairs
        sit at partition distance 32, so the swap is between 32-blocks
        (0<->1, 2<->3). s_sb rows carry the sign: [-sin, sin, -sin, sin].
        Tables are pre-scaled by 1/WSCALE so the result is true-scale.
        """
        st_bf = rtmp.tile([P, 512], BF16, tag="st", name="st_bf")
        nc.vector.tensor_copy(st_bf[:], ps[:])
        xsw = rtmp.tile([P, 512], BF16, tag="xsw", name="xsw")
        for blk in range(4):
            sb = blk ^ 1
            nc.vector.tensor_copy(xsw[blk * 32:(blk + 1) * 32, :],
                                  st_bf[sb * 32:(sb + 1) * 32, :])
        nc.vector.tensor_mul(xsw[:], xsw[:], s_sb[:, ssl])
        t2 = rtmp.tile([P, 512], BF16, tag="t2", name="t2")
        nc.vector.tensor_mul(t2[:], st_bf[:], c_sb[:, ssl])
        nc.vector.tensor_add(dst, t2[:], xsw[:])

    def proj_chunk(w_a, x_a, t, nm):
        # fp8 DoubleRow residual projection: w8@x8 + wr8@x8 + w8@xr8,
        # 12 DR matmuls (et-pairs) accumulating into one [P,512] group.
        ps = psum_p.tile([P, 512], F32, tag="pp", name=f"pp_{nm}")
        tsl = slice(t * P, (t + 1) * P)
        terms = ((0, 0), (1, 0), (0, 1))
        for ti, (wr, xr) in enumerate(terms):
            for p8 in range(NET // 2):
                esl = slice(2 * p8, 2 * p8 + 2)
                nc.tensor.matmul(
                    ps[:], w_a[:, wr, esl, tsl], x_a[:, xr, esl, :],
                    start=(ti == 0 and p8 == 0),
                    stop=(ti == 2 and p8 == NET // 2 - 1),
                    perf_mode=DRM,
                )
        return ps

    krot = [krot_pool.tile([P, S], BF16, tag="krot", name=f"krot{i}")
            for i in range(NDT)]
    vreg = vreg_pool.tile([P, NST, HPC * 64], BF16, tag="vreg")
    vreg_v = vreg.rearrange("p st (h dd) -> p st h dd", h=HPC)

    def kproj_piece(t, sc):
        ssl = slice(sc * 512, (sc + 1) * 512)
        ps = proj_chunk(wk_a, ktc_a[:, :, sc], t, f"k{t}{sc}")
        rope_evict(ps, krot[t][:, ssl], ssl)

    qp_state = {}

    def qproj_load(qc):
        qsl = slice(qc * 512, (qc + 1) * 512)
        xq_a = xq.tile([P, 2, NET, 512], FP8, tag="xqp", name=f"x_q{qc}")
        for r in range(2):
            nc.gpsimd.dma_start(
                out=xq_a[:, r],
                in_=q8[r, :, qsl].rearrange("(a p) c -> p a c", p=P))
        qr = qrot_pool.tile([P, NDT, 512], BF16, tag="qrot", name=f"qr{qc}")
        qp_state[qc] = (xq_a, qr)
        return qr

    def qproj_piece(qc, t):
        xq_a, qr = qp_state[qc]
        qsl = slice(qc * 512, (qc + 1) * 512)
        ps = proj_chunk(wq_a, xq_a, t, f"q{qc}{t}")
        rope_evict(ps, qr[:, t, :], qsl)

    vload_pending = {}

    def vload(sc):
        # prefetch the vT chunk one scores-chunk ahead of its vproj
        xv_a = xv.tile([P, 2, NET, 512], FP8, tag="xv", name=f"x_v{sc}")
        for r in range(2):
            nc.gpsimd.dma_start(
                out=xv_a[:, r],
                in_=v8[r, :, sc * 512:(sc + 1) * 512].rearrange(
                    "(a p) c -> p a c", p=P))
        vload_pending[sc] = xv_a

    def vproj_st(sc, sti):
        # V for all 8 heads over one s-tile of chunk sc; DR residual with
        # x as the stationary operand (out [s, hd]); evict with 1/WSCALE.
        xv_a = vt0 if sc == 0 else vload_pending[sc]
        st = sc * 4 + sti
        ps = psum_p.tile([P, 512], F32, tag="pp", name=f"pp_v{sc}{st}")
        ssl = slice(sti * P, (sti + 1) * P)
        terms = ((0, 0), (1, 0), (0, 1))
        for ti, (xr, wr) in enumerate(terms):
            for p8 in range(NET // 2):
                esl = slice(2 * p8, 2 * p8 + 2)
                nc.tensor.matmul(
                    ps[:], xv_a[:, xr, esl, ssl], wv_a[:, wr, esl, :],
                    start=(ti == 0 and p8 == 0),
                    stop=(ti == 2 and p8 == NET // 2 - 1),
                    perf_mode=DRM,
                )
        nc.vector.tensor_scalar_mul(
            vreg_v[:, st, :, 0:64],
            ps[:].rearrange("p (h d) -> p h d", h=HPC),
            1.0 / WSCALE,
        )

    def outproj_st(qc, aall, sti):
        # output projection for one s-tile of chunk qc (bf16, unchanged)
        st = qc * 4 + sti
        osb = opool.tile([P, E], F32, tag="osb", name=f"osb{st}")
        for ec in range(2):
            esl = slice(ec * 512, (ec + 1) * 512)
            ps_f = psum_p.tile([P, 512], F32, tag="pp", name=f"pp_f{st}{ec}")
            for t in range(NDT):
                nc.tensor.matmul(ps_f[:],
                                 aall[:, t, sti * P:(sti + 1) * P],
                                 wo_sb[t][:, esl],
                                 start=(t == 0), stop=(t == NDT - 1))
            nc.vector.tensor_copy(osb[:, esl], ps_f[:])
        nc.sync.dma_start(out=out[st * P:(st + 1) * P, :], in_=osb[:])

    # ---------------- attention, software-pipelined per q-chunk -------------
    qr0 = qrot_pool.tile([P, NDT, 512], BF16, tag="qrot", name="qr0")

    def qproj0_piece(t):
        ps = proj_chunk(wq_a, xq0, t, f"q0{t}")
        rope_evict(ps, qr0[:, t, :], slice(0, 512))

    kproj_piece(0, 0)
    qproj0_piece(0)

    qr_cur = qr0
    aall_prev = None
    aall = None
    asb_prev = None
    asb = None
    pending = None  # (t, ktq, ptt, asb) attnv chunk awaiting emission
    av_state = {}   # half H -> (av_tiles[2], d_tile)
    half_ctr = [0]

    def start_half(H):
        n = half_ctr[0]
        half_ctr[0] += 1
        avs = [psum_av.tile([P, 2, 4, 64], F32, tag="av", name=f"av{n}_{i}")
               for i in range(2)]
        d_t = psum_d.tile([P, 16], F32, tag="d", name=f"d{n}")
        av_state[H] = (avs, d_t)

    def emit_attnv_part(t, ktq, ptt, j):
        # q-major attnv + denominator for one kt of a pending chunk
        H = t // 2
        if t % 2 == 0 and ktq == 0 and j == 0:
            start_half(H)
        avs, d_t = av_state[H]
        first = (t % 2 == 0 and ktq == 0 and j == 0)
        last = (t % 2 == 1 and ktq == NST // NKQ - 1 and j == NKQ - 1)
        kt = ktq * NKQ + j
        for qt in range(4):
            qsl = slice(qt * P, (qt + 1) * P)
            av = avs[qt // 2]
            for gl in range(2):
                hl = 2 * (t % 2) + gl
                hg = 2 * t + gl
                nc.tensor.matmul(
                    av[:, qt % 2, hl, :],
                    ptt[:, j, gl, qsl], vreg_v[:, kt, hg, :],
                    start=(first and qt % 2 == 0 and gl == 0),
                    stop=(last and qt % 2 == 1 and gl == 1),
                )
                nc.tensor.matmul(
                    d_t[:, qt * 4 + hl:qt * 4 + hl + 1],
                    ptt[:, j, gl, qsl], ones[:, 0:1],
                    start=(first and qt == 0 and gl == 0),
                    stop=(last and qt == 3 and gl == 1),
                )

    def finish_half(H, asb_t):
        # reciprocal of the 16 denominators, then evict+normalize the 16
        # [128,64] blocks of the half into asb (bf16, [q, hd])
        avs, d_t = av_state.pop(H)
        rec = rec_pool.tile([P, 16], F32, tag="rec",
                            name=f"rec{half_ctr[0]}")
        nc.vector.reciprocal(rec[:], d_t[:])
        for qt in range(4):
            av = avs[qt // 2]
            for hl in range(4):
                hg = 4 * H + hl
                nc.vector.tensor_scalar_mul(
                    asb_t[:, qt, hg * 64:(hg + 1) * 64],
                    av[:, qt % 2, hl, :],
                    rec[:, qt * 4 + hl:qt * 4 + hl + 1],
                )

    def emit_attnv(t, ktq, ptt, skip_parts=0):
        for j in range(skip_parts, NKQ):
            emit_attnv_part(t, ktq, ptt, j)

    def transpose_qt(asb_t, qt, aall_t):
        # asb [q, hd] -> aall [hd, q] via PE transpose, 4 [128,128] pieces
        for tt in range(NDT):
            pst = psum_p.tile([P, P], BF16, tag="pp", name=f"tp{qt}{tt}")
            nc.tensor.transpose(pst[:], asb_t[:, qt, tt * P:(tt + 1) * P],
                                id_sb[:])
            nc.vector.tensor_copy(aall_t[:, tt, qt * P:(qt + 1) * P], pst[:])

    for qc in range(NSC):
        aall_prev = aall
        asb_prev = asb
        aall = aall_pool.tile([P, NDT, 512], BF16, tag="aall", name=f"aall{qc}")
        asb = asb_pool.tile([P, 4, 512], BF16, tag="asb", name=f"asb{qc}")
        for t in range(NDT):
            Kt = krot[t]
            # next q-projection piece at slot start: ready well before its
            # own stream, without loading qc0 (whose pieces stagger via
            # qproj0 below)
            if qc + 1 < NSC and t == 0:
                qr_next = qproj_load(qc + 1)
            if qc >= 1:
                if t + 1 < NDT:
                    qproj_piece(qc, t + 1)
                elif qc + 1 < NSC:
                    qproj_piece(qc + 1, 0)
            for ktq in range(NST // NKQ):
                if qc == 0 and (ktq * NKQ) % 4 == 0:
                    sc = ktq * NKQ // 4
                    if t == 0:
                        # stream the remaining kT/vT chunks just ahead of use
                        if sc + 2 < NSC:
                            for r in range(2):
                                nc.gpsimd.dma_start(
                                    out=ktc_a[:, r, sc + 2],
                                    in_=k8[r, :, (sc + 2) * 512:(sc + 3) * 512]
                                    .rearrange("(a p) c -> p a c", p=P))
                        if sc + 1 < NSC:
                            vload(sc + 1)
                    # prefetch the next K chunk / next tile's first pieces
                    if sc + 1 < NSC:
                        kproj_piece(t, sc + 1)
                    elif t + 1 < NDT:
                        qproj0_piece(t + 1)
                        kproj_piece(t + 1, 0)
                    elif qc + 1 < NSC:
                        qproj_piece(qc + 1, 0)
                ptt = pt_pool.tile([P, NKQ, 2, 512], BF16, tag="pt",
                                   name=f"pt{qc}_{t}_{ktq}")
                for j in range(NKQ):
                    kt = ktq * NKQ + j
                    ksl = slice(kt * P, (kt + 1) * P)
                    psS = psum_s.tile([P, 2, 512], F32, tag="ps",
                                      name=f"psS{qc}{t}{kt}")
                    nc.tensor.matmul(psS[:, 0, :], Kt[0:64, ksl],
                                     qr_cur[0:64, t, :], start=True, stop=True,
                                     tile_position=(0, 0))
                    nc.tensor.matmul(psS[:, 1, :], Kt[64:128, ksl],
                                     qr_cur[64:128, t, :], start=True, stop=True,
                                     tile_position=(64, 0))
                    nc.scalar.activation(ptt[:, j, :, :], psS[:],
                                         mybir.ActivationFunctionType.Exp,
                                         scale=0.125)
                    if pending is not None and j > 0:
                        # spread the pending chunk between the scores pairs
                        emit_attnv_part(pending[0], pending[1], pending[2],
                                        j - 1)
                    if qc == 0 and t == 0:
                        # V-projection s-tiles interleaved with the scores
                        # stream, one chunk ahead of their attnv consumers
                        vproj_st(ktq, j)
                if pending is not None:
                    # last part of the pending chunk (+ half finish if due)
                    pt_, ktq_, ptt_, asb_ = pending
                    emit_attnv_part(pt_, ktq_, ptt_, NKQ - 1)
                    if pt_ % 2 == 1 and ktq_ == NST // NKQ - 1:
                        finish_half(pt_ // 2, asb_)
                pending = (t, ktq, ptt, asb)
            # PE filler between d-tiles: previous-chunk transposes + output
            # s-tile.
            if qc >= 1:
                transpose_qt(asb_prev, t, aall_prev)
                outproj_st(qc - 1, aall_prev, t)
        if qc + 1 < NSC:
            qr_cur = qr_next
    # drain the last attnv chunk, finish the last half, final transposes +
    # output projections
    if pending is not None:
        pt_, ktq_, ptt_, asb_ = pending
        emit_attnv(pt_, ktq_, ptt_, skip_parts=0)
        finish_half(pt_ // 2, asb_)
        pending = None
    for sti in range(4):
        transpose_qt(asb, sti, aall)
        outproj_st(NSC - 1, aall, sti)


# ---------------------------------------------------------------------------
# host side
# ---------------------------------------------------------------------------

_PROGRAM = None


def _get_program():
    global _PROGRAM
    if _PROGRAM is None:
        _PROGRAM = build_program()
    return _PROGRAM


def _np_bf16():
    import ml_dtypes

    return np.dtype(ml_dtypes.bfloat16)


def _np_fp8():
    import ml_dtypes

    return np.dtype(ml_dtypes.float8_e4m3)


def _fp8_pair(a):
    """a (f32) -> stacked [2, ...] fp8e4m3 (main, residual)."""
    dt8 = _np_fp8()
    a8 = a.astype(dt8)
    r8 = (a - a8.astype(np.float32)).astype(dt8)
    return np.ascontiguousarray(np.stack([a8, r8], axis=0))


def _perm_rows(hh):
    """Row permutation of Wq/Wk for one head-half.

    Per head h: [h evens (32) | h odds (32)], heads consecutive. Within a
    128-tile t: head 2t rows 0:64, head 2t+1 rows 64:128; RoPE pairs sit at
    partition distance 32 inside each head's 64 rows.
    """
    base = hh * HPC * D
    rows = []
    for h in range(HPC):
        a = base + h * D
        rows += [a + 2 * i for i in range(32)]
        rows += [a + 2 * i + 1 for i in range(32)]
    return np.array(rows, dtype=np.int64)


def _tables():
    inv_freq = 1.0 / (10000.0 ** (np.arange(0, D, 2, dtype=np.float32) / D))
    freqs = np.arange(S, dtype=np.float32)[:, None] * inv_freq[None, :]  # [S, 32]
    cos = np.cos(freqs).T.astype(np.float32) / WSCALE  # [32, S]
    sin = np.sin(freqs).T.astype(np.float32) / WSCALE
    C = np.tile(cos, (4, 1))  # [128, S]
    Ssig = np.concatenate([-sin, sin, -sin, sin], axis=0).astype(np.float32)
    dt = _np_bf16()
    return np.ascontiguousarray(C).astype(dt), np.ascontiguousarray(Ssig).astype(dt)


def prepare_inputs(query, key, value, Wq, Wk, Wv, Wo, bo):
    dt = _np_bf16()
    C, Ssig = _tables()
    ident = np.eye(P, dtype=np.float32).astype(dt)
    xTs = {}
    for b in range(B):
        xTs[b] = tuple(
            _fp8_pair(np.ascontiguousarray(np.asarray(x[b], np.float32).T))
            for x in (query, key, value)
        )
    per_hh = {}
    for hh in range(2):
        perm = _perm_rows(hh)
        dsl = slice(hh * DPC, (hh + 1) * DPC)
        per_hh[hh] = {
            "wq8": _fp8_pair(
                np.ascontiguousarray(np.asarray(Wq, np.float32)[perm, :].T) * WSCALE),
            "wk8": _fp8_pair(
                np.ascontiguousarray(np.asarray(Wk, np.float32)[perm, :].T) * WSCALE),
            "wv8": _fp8_pair(
                np.ascontiguousarray(np.asarray(Wv, np.float32)[dsl, :].T) * WSCALE),
            "woT": np.ascontiguousarray(
                np.asarray(Wo, np.float32)[:, dsl].T).astype(dt),
        }
    in_maps = []
    for c in range(8):
        b, hh = c // 2, c % 2
        q8b, k8b, v8b = xTs[b]
        m = {"q8": q8b, "k8": k8b, "v8": v8b, "ctab": C, "stab": Ssig,
             "ident": ident}
        m.update(per_hh[hh])
        in_maps.append(m)
    return in_maps


def kernel(query, key, value, Wq, Wk, Wv, Wo, bo):
    nc = _get_program()
    in_maps = prepare_inputs(query, key, value, Wq, Wk, Wv, Wo, bo)
    res = run_bass_kernel_spmd(nc, in_maps, list(range(8)))
    bo = np.asarray(bo, np.float32)
    out = np.empty((B, S, E), np.float32)
    for b in range(B):
        out[b] = res.results[b * 2]["out"] + res.results[b * 2 + 1]["out"] + bo
    return out


if __name__ == "__main__":
    rng = np.random.default_rng(0)
    ins = {
        "query": rng.standard_normal((B, S, E)).astype(np.float32),
        "key": rng.standard_normal((B, S, E)).astype(np.float32),
        "value": rng.standard_normal((B, S, E)).astype(np.float32),
        "Wq": (rng.standard_normal((E, E)) * 0.02).astype(np.float32),
        "Wk": (rng.standard_normal((E, E)) * 0.02).astype(np.float32),
        "Wv": (rng.standard_normal((E, E)) * 0.02).astype(np.float32),
        "Wo": (rng.standard_normal((E, E)) * 0.02).astype(np.float32),
        "bo": np.zeros((E,), np.float32),
    }
    o = kernel(**ins)
    print("out", o.shape, o.dtype, float(np.abs(o).max()))
